# revision 28
# baseline (speedup 1.0000x reference)
"""Causal multi-head attention on 8 trn2 NeuronCores.

Sharding: core c -> (batch b = c//2, head-group hg = c%2).
Each head-group owns 8 of the 16 heads (512 of the 1024 embed dims after
the head split).

v3 layout (all matmul operands bf16, PSUM accumulation fp32):
  - qT, kT = (x[b] @ Wq_hg)^T, (x[b] @ Wk_hg)^T    [cols, rows] bf16
    (softmax 1/sqrt(d) scale folded into Wq on host)
  - v packed as va [rows, 8*(64+1)] bf16 with a ones column per head so
    the attn@V matmul also produces the softmax denominator (row 64).
  - scoresT [k, q] per (head, 512-q-chunk, 128-k-tile); exp -> bf16 on
    the Act engine; causal-diagonal tiles then have their first 128
    columns multiplied by a binary mask on the DVE (fast 2-byte mode).
  - normalize: denom row -> SBUF -> reciprocal_approx_fast (DVE) ->
    partition_broadcast (GPSIMD) -> one DVE multiply into ctxT bf16.
  - partial = ctxT.T @ Wproj_hg (+ bproj on hg==0 cores).
Host: out[b] = partial(b,0) + partial(b,1).

Emission order is software-pipelined: score matmuls run 3 k-tiles ahead
of the attn@V matmuls (so PE never waits on exp), the q/k projections
for later head-groups are interleaved into the attention stream (keeps
the PE HAM clock-gate at full speed), and DMA traffic is spread over
the sync/scalar/gpsimd queues so the lead-in is not serialized on one
engine. A burst of dummy matmuls on the first-arriving const tile
un-throttles the HAM clock gate (1.2 -> 2.4 GHz) during the load phase.
"""

import sys

try:
    import concourse.bass as bass  # noqa: F401
except Exception:
    sys.path.insert(0, "/opt/trn_rl_repo")

import ml_dtypes
import numpy as np

import concourse.bass as bass
import concourse.mybir as mybir
import concourse.tile as tile
from concourse import bacc
from concourse.bass_utils import run_bass_kernel_spmd

F32 = mybir.dt.float32
F32R = mybir.dt.float32r
BF16 = mybir.dt.bfloat16
AF = mybir.ActivationFunctionType
BF = ml_dtypes.bfloat16

B, S, E = 4, 1024, 1024
H, D = 16, 64
HG = 2              # head groups (cores per batch)
HPG = H // HG       # 8 heads per group
EG = HPG * D        # 512 embed cols per group
P = 128
ET = E // P         # 8 embed tiles
RT = S // P         # 8 row tiles
CT = EG // P        # 4 col tiles of the group's q/k
QCH = 512           # q-chunk (moving free dim; ISA max for fp32 PSUM out)
NQC = S // QCH      # 2 q chunks
KTQ = QCH // P      # 4 k-tiles per q chunk
SCALE = 1.0 / np.sqrt(D)


def _emit(nc, tc, with_bias):
    # inputs pre-packed on host into [128, *] layouts with long
    # contiguous rows so each loads as ONE descriptor-light DMA
    xT = nc.dram_tensor("xT", [P, ET * S], BF16, kind="ExternalInput")
    wq = nc.dram_tensor("wq", [P, ET * EG], BF16, kind="ExternalInput")
    wk = nc.dram_tensor("wk", [P, ET * EG], BF16, kind="ExternalInput")
    wv = nc.dram_tensor("wv", [P, ET * EG], BF16, kind="ExternalInput")
    wp = nc.dram_tensor("wp", [P, CT * E], BF16, kind="ExternalInput")
    # packed constants: cb = binary causal mask(128) | vones(8)  (bf16)
    cb = nc.dram_tensor("cb", [P, P + HPG], BF16, kind="ExternalInput")
    # bqk = bq(4) | bk(4)  (f32, per-partition bias)
    bqk = nc.dram_tensor("bqk", [P, 2 * CT], F32, kind="ExternalInput")
    # crow = ones(512) | bv(512) | bp(1024)  (f32 rows)
    crow = nc.dram_tensor("crow", [1, QCH + EG + E], F32,
                          kind="ExternalInput")
    out = nc.dram_tensor("out", [S, E], BF16, kind="ExternalOutput")

    with (
        tc.tile_pool(name="big", bufs=1) as p_big,
        tc.tile_pool(name="exs", bufs=6) as p_ex,
        tc.tile_pool(name="rc", bufs=4) as p_rc,
        tc.tile_pool(name="rcb", bufs=4) as p_rcb,
        tc.tile_pool(name="osb", bufs=4) as p_osb,
        tc.tile_pool(name="sm", bufs=1) as p_sm,
        tc.tile_pool(name="sc", bufs=3, space="PSUM") as p_sc,
        tc.tile_pool(name="qk", bufs=2, space="PSUM") as p_qk,
        tc.tile_pool(name="avp", bufs=3, space="PSUM") as p_av,
    ):
        # ---- constants: packed DMAs on the gpsimd queue ----
        cb_sb = p_sm.tile([P, P + HPG], BF16, tag="cb", name="cbt")
        nc.gpsimd.dma_start(cb_sb[:], cb[:])
        mask_sb = cb_sb[:, 0:P]
        vones_sb = cb_sb[:, P:P + HPG]
        bqk_sb = p_sm.tile([P, 2 * CT], F32, tag="bqk", name="bqkt")
        nc.gpsimd.dma_start(bqk_sb[:], bqk[:])
        bq_sb = bqk_sb[:, 0:CT]
        bk_sb = bqk_sb[:, CT:2 * CT]
        crow_sb = p_sm.tile([1, QCH + EG + E], F32, tag="crow", name="crowt")
        nc.gpsimd.dma_start(crow_sb[:], crow[:])
        ones_sb = crow_sb[:, 0:QCH].bitcast(F32R)
        bv_sb = crow_sb[:, QCH:QCH + EG].bitcast(F32R)
        bp_sb = crow_sb[:, QCH + EG:].bitcast(F32R)

        # ---- persistent sbuf tiles ----
        xt_b = p_big.tile([P, ET * S], BF16, tag="xtb", name="xtb")
        xt_t = [xt_b[:, et * S:(et + 1) * S] for et in range(ET)]
        wq_b = p_big.tile([P, ET * EG], BF16, tag="wqb", name="wqb")
        wq_t = [wq_b[:, et * EG:(et + 1) * EG] for et in range(ET)]
        wk_b = p_big.tile([P, ET * EG], BF16, tag="wkb", name="wkb")
        wk_t = [wk_b[:, et * EG:(et + 1) * EG] for et in range(ET)]
        wv_b = p_big.tile([P, ET * EG], BF16, tag="wvb", name="wvb")
        wv_t = [wv_b[:, et * EG:(et + 1) * EG] for et in range(ET)]
        wp_b = p_big.tile([P, CT * E], BF16, tag="wpb", name="wpb")
        wp_t = [wp_b[:, et * E:(et + 1) * E] for et in range(CT)]
        qT_t = [p_big.tile([P, S], BF16, tag=f"qt{ct}", name=f"qt{ct}")
                for ct in range(CT)]
        kT_t = [p_big.tile([P, S], BF16, tag=f"kt{ct}", name=f"kt{ct}")
                for ct in range(CT)]
        va_t = [p_big.tile([P, HPG * (D + 1)], BF16, tag=f"va{rt}",
                           name=f"va{rt}") for rt in range(RT)]
        ctx_t = [p_big.tile([P, S], BF16, tag=f"cx{ct}", name=f"cx{ct}")
                 for ct in range(CT)]

        # ---- input DMA: consumption-ordered trios ----
        # each et's (xt first-half, wq, wk) lands in parallel across the
        # three DMA-capable queues (sync/scalar/gpsimd), so the q0/k0
        # projections chase arrivals with no cross-tensor queuing delay;
        # xt second halves + wv follow, wp last
        engs = [nc.sync, nc.scalar, nc.gpsimd]
        for et in range(ET):
            engs[et % 3].dma_start(
                xt_t[et][:, 0:QCH], xT[:, et * S:et * S + QCH])
            engs[(et + 1) % 3].dma_start(
                wq_t[et], wq[:, et * EG:(et + 1) * EG])
            engs[(et + 2) % 3].dma_start(
                wk_t[et], wk[:, et * EG:(et + 1) * EG])
        for et in range(ET):
            engs[et % 3].dma_start(
                xt_t[et][:, QCH:S], xT[:, et * S + QCH:(et + 1) * S])
            engs[(et + 1) % 3].dma_start(
                wv_t[et], wv[:, et * EG:(et + 1) * EG])
        for et in range(CT):
            engs[et % 3].dma_start(wp_t[et], wp[:, et * E:(et + 1) * E])

        # ---- q/k projection chunk: qT/kT[ct][:, rc*QCH:+QCH] ----
        def emit_qk_chunk(dst, w_t, b_sb, ct, rc, engine, fill=0):
            ps = p_qk.tile([P, QCH], F32, tag="qk", name="qk")
            for et in range(ET):
                nc.tensor.matmul(
                    ps[:],
                    w_t[et][:, ct * P:(ct + 1) * P],
                    xt_t[et][:, rc * QCH:(rc + 1) * QCH],
                    start=(et == 0), stop=(et == ET - 1),
                )
                warm(fill)
            dst_ap = dst[ct][:, rc * QCH:(rc + 1) * QCH]
            if with_bias:
                nc.scalar.activation(
                    dst_ap, ps[:], AF.Identity, bias=b_sb[:, ct:ct + 1])
            elif engine == "act":
                nc.scalar.activation(dst_ap, ps[:], AF.Copy)
            else:
                nc.vector.tensor_copy(dst_ap, ps[:])

        # ---- v projection + augmented-va pack for one row tile ----
        def emit_v_rt(rt, fill=0):
            va3 = va_t[rt][:].rearrange("p (h d) -> p h d", h=HPG)
            nc.vector.tensor_copy(
                va3[:, :, D:D + 1],
                vones_sb.rearrange("p (h o) -> p h o", o=1))
            ps = p_qk.tile([P, QCH], F32, tag="qk", name="qk")
            for et in range(ET):
                nc.tensor.matmul(
                    ps[:, 0:EG],
                    xt_t[et][:, rt * P:(rt + 1) * P],
                    wv_t[et][:],
                    start=(et == 0),
                    stop=(et == ET - 1 and not with_bias),
                )
                warm(fill)
            if with_bias:
                nc.tensor.matmul(
                    ps[:, 0:EG], ones_sb[0:1, 0:P], bv_sb[0:1, :],
                    start=False, stop=True,
                )
            ps3 = ps[:, 0:EG].rearrange("p (h d) -> p h d", h=HPG)
            nc.vector.tensor_copy(va3[:, :, 0:D], ps3[:])

        # ---- HAM warmup: ~4us of continuous dummy matmuls on the
        # first-arriving (tiny) const tile un-throttles the PE clock
        # gate (1.2 -> 2.4 GHz) before the real lead runs. Results are
        # discarded: the first real matmul's start=True clears PSUM.
        avw = p_av.tile([D + 1, QCH], F32, tag="av", name="av")
        warm_first = [True]

        def warm(n):
            for _ in range(n):
                nc.tensor.matmul(
                    avw[0:D, 0:P], mask_sb[:, 0:D], mask_sb,
                    start=warm_first[0], stop=False, skip_group_check=True,
                )
                warm_first[0] = False

        warm(44)

        # ---- lead phase (minimal): unit (h0,qc0) only needs the rc0
        # halves of q/k ct0 plus va0-3; everything else is injected into
        # the attention stream (converts on DVE: the Act engine's queue
        # is busy issuing wq/wv DMAs in this window) ----
        emit_qk_chunk(qT_t, wq_t, bq_sb, 0, 0, "dve")
        emit_qk_chunk(kT_t, wk_t, bk_sb, 0, 0, "dve")
        for rt in range(4):
            emit_v_rt(rt)

        # remaining q/k chunks + v row tiles, injected mid-attention:
        # dense full-K PE work that keeps the HAM clock-gate warm while
        # the queued exps keep the Act engine busy. Each chunk must land
        # one unit before its first consumer.
        def qk_inj(dst, w_t, b_sb, ct, rc):
            return lambda: emit_qk_chunk(dst, w_t, b_sb, ct, rc, "dve")

        inj = {0: [qk_inj(qT_t, wq_t, bq_sb, 0, 1),
                   qk_inj(kT_t, wk_t, bk_sb, 0, 1),
                   lambda: emit_v_rt(4),
                   lambda: emit_v_rt(5)],
               1: [lambda: emit_v_rt(6),
                   lambda: emit_v_rt(7)]}
        u = 2
        for ct in range(1, CT):
            inj[u] = [qk_inj(qT_t, wq_t, bq_sb, ct, 0)]
            inj[u + 1] = [qk_inj(kT_t, wk_t, bk_sb, ct, 0)]
            inj[u + 2] = [qk_inj(qT_t, wq_t, bq_sb, ct, 1)]
            inj[u + 3] = [qk_inj(kT_t, wk_t, bk_sb, ct, 1)]
            u += 4

        # ---- attention ----
        for h in range(HPG):
            hp, hb = h // 2, (h % 2) * D
            va3s = [va_t[kt][:].rearrange("p (h d) -> p h d", h=HPG)[:, h, :]
                    for kt in range(RT)]
            for qc in range(NQC):
                n_kt = (qc + 1) * KTQ
                av = p_av.tile([D + 1, QCH], F32, tag="av", name="av")
                exs = {}

                def emit_sc(kt, qc=qc, exs=exs):
                    off = max(0, kt - qc * KTQ) * P
                    n = QCH - off
                    diag = (qc == 0) or (kt >= KTQ)
                    sc = p_sc.tile([P, QCH], F32, tag="sc", name="sc")
                    nc.tensor.matmul(
                        sc[:, 0:n],
                        kT_t[hp][hb:hb + D, kt * P:(kt + 1) * P],
                        qT_t[hp][hb:hb + D,
                                 qc * QCH + off:(qc + 1) * QCH],
                        start=True, stop=True,
                        tile_position=(hb, 0),
                    )
                    ex = p_ex.tile([P, QCH], BF16, tag="ex", name="ex")
                    nc.scalar.activation(ex[:, 0:n], sc[:, 0:n], AF.Exp)
                    if diag:
                        nc.vector.tensor_mul(
                            ex[:, 0:P], ex[:, 0:P], mask_sb)
                    exs[kt] = (ex, off, n)

                LOOK = 3
                for kt in range(min(LOOK, n_kt)):
                    emit_sc(kt)
                # inject projection chunks mid-unit: the queued exps
                # keep the Act engine busy while PE runs them
                for fn in inj.get(h * NQC + qc, ()):
                    fn()
                for kt in range(n_kt):
                    ex, off, n = exs.pop(kt)
                    nc.tensor.matmul(
                        av[:, off:QCH],
                        va3s[kt],
                        ex[:, 0:n],
                        start=(kt == 0), stop=(kt == n_kt - 1),
                    )
                    if kt + LOOK < n_kt:
                        emit_sc(kt + LOOK)

                # normalize: all off the PE stream
                # (reciprocal_approx_fast's bitwise seed misreads PSUM,
                # so stage the denominator row through SBUF first)
                dn_sb = p_rc.tile([1, QCH], F32, tag="dn", name="dn")
                nc.vector.tensor_copy(dn_sb[:], av[D:D + 1, :])
                rc_sb = p_rc.tile([1, QCH], F32, tag="rc", name="rc")
                nc.vector.reciprocal_approx_fast(rc_sb[:], dn_sb[:])
                rcb = p_rcb.tile([D, QCH], F32, tag="rcb", name="rcb")
                nc.gpsimd.partition_broadcast(rcb[:], rc_sb[:], channels=D)
                nc.vector.tensor_mul(
                    ctx_t[hp][hb:hb + D, qc * QCH:(qc + 1) * QCH],
                    av[0:D, :], rcb[:])

        # ---- output projection: partial = ctxT.T @ wp (+ bp) ----
        osb_eng = 0
        for rt in range(RT):
            for cc in range(E // QCH):
                ps = p_sc.tile([P, QCH], F32, tag="sc", name="sc")
                for et in range(CT):
                    nc.tensor.matmul(
                        ps[:],
                        ctx_t[et][:, rt * P:(rt + 1) * P],
                        wp_t[et][:, cc * QCH:(cc + 1) * QCH],
                        start=(et == 0),
                        stop=(et == CT - 1 and not with_bias),
                    )
                if with_bias:
                    nc.tensor.matmul(
                        ps[:], ones_sb[0:1, 0:P],
                        bp_sb[0:1, cc * QCH:(cc + 1) * QCH],
                        start=False, stop=True,
                    )
                osb = p_osb.tile([P, QCH], BF16, tag="osb", name="osb")
                if osb_eng == 0:
                    nc.vector.tensor_copy(osb[:], ps[:])
                else:
                    nc.scalar.activation(osb[:], ps[:], AF.Copy)
                osb_eng = (osb_eng + 1) % 2
                dma_eng = [nc.sync, nc.gpsimd, nc.scalar][
                    (rt * 2 + cc) % 3]
                dma_eng.dma_start(
                    out[rt * P:(rt + 1) * P, cc * QCH:(cc + 1) * QCH],
                    osb[:])


def build_nc(with_bias=False):
    nc = bacc.Bacc("TRN2", target_bir_lowering=False, debug=False)
    with tile.TileContext(nc) as tc, nc.allow_low_precision(
        reason="bf16 matmul pipeline; fp32 PSUM accumulate"
    ):
        _emit(nc, tc, with_bias)
    nc.compile()
    return nc


def make_in_maps(x, Wqkv, bqkv, Wproj, bproj):
    x = np.asarray(x, dtype=np.float32)
    Wqkv = np.asarray(Wqkv, dtype=np.float32)
    bqkv = np.asarray(bqkv, dtype=np.float32)
    Wproj = np.asarray(Wproj, dtype=np.float32)
    bproj = np.asarray(bproj, dtype=np.float32)
    keep = np.triu(np.ones((P, P), dtype=np.float32))  # [k, q]: k <= q
    cb = np.concatenate([
        keep,                                 # binary causal mask
        np.ones((P, HPG), dtype=np.float32),  # vones
    ], axis=1).astype(BF)
    in_maps = []
    for c in range(8):
        b, hg = c // 2, c % 2
        g = slice(hg * EG, (hg + 1) * EG)
        bqk = np.concatenate([
            (bqkv[0 * E:1 * E][g] * SCALE).reshape(CT, P).T,
            bqkv[1 * E:2 * E][g].reshape(CT, P).T], axis=1)
        crow = np.concatenate([
            np.ones(QCH, dtype=np.float32),
            bqkv[2 * E:3 * E][g],
            bproj if hg == 0 else np.zeros_like(bproj),
        ]).reshape(1, QCH + EG + E)
        def pack(a):
            # [n*128, m] -> [128, n*m] with row-major et-chunks
            n = a.shape[0] // P
            return np.ascontiguousarray(
                a.reshape(n, P, a.shape[1]).transpose(1, 0, 2)
                .reshape(P, n * a.shape[1])).astype(BF)
        in_maps.append({
            "xT": pack(x[b].T),
            "wq": pack(Wqkv[:, 0 * E:1 * E][:, g] * SCALE),
            "wk": pack(Wqkv[:, 1 * E:2 * E][:, g]),
            "wv": pack(Wqkv[:, 2 * E:3 * E][:, g]),
            "wp": pack(Wproj[g, :]),
            "cb": cb,
            "bqk": np.ascontiguousarray(bqk),
            "crow": np.ascontiguousarray(crow),
        })
    return in_maps


def kernel(x, Wqkv, bqkv, Wproj, bproj):
    with_bias = bool(
        np.any(np.asarray(bqkv)) or np.any(np.asarray(bproj)))
    nc = build_nc(with_bias)
    in_maps = make_in_maps(x, Wqkv, bqkv, Wproj, bproj)
    res = run_bass_kernel_spmd(nc, in_maps, list(range(8))).results
    out = np.zeros((B, S, E), dtype=np.float32)
    for c in range(8):
        out[c // 2] += res[c]["out"].astype(np.float32)
    return out


# revision 29
# speedup vs baseline: 1.0142x; 1.0142x over previous
"""Causal multi-head attention on 8 trn2 NeuronCores.

Sharding: core c -> (batch b = c//2, head-group hg = c%2).
Each head-group owns 8 of the 16 heads (512 of the 1024 embed dims after
the head split).

v3 layout (all matmul operands bf16, PSUM accumulation fp32):
  - qT, kT = (x[b] @ Wq_hg)^T, (x[b] @ Wk_hg)^T    [cols, rows] bf16
    (softmax 1/sqrt(d) scale folded into Wq on host)
  - v packed as va [rows, 8*(64+1)] bf16 with a ones column per head so
    the attn@V matmul also produces the softmax denominator (row 64).
  - scoresT [k, q] per (head, 512-q-chunk, 128-k-tile); exp -> bf16 on
    the Act engine; causal-diagonal tiles then have their first 128
    columns multiplied by a binary mask on the DVE (fast 2-byte mode).
  - normalize: denom row -> SBUF -> reciprocal_approx_fast (DVE) ->
    partition_broadcast (GPSIMD) -> one DVE multiply into ctxT bf16.
  - partial = ctxT.T @ Wproj_hg (+ bproj on hg==0 cores).
Host: out[b] = partial(b,0) + partial(b,1).

Emission order is software-pipelined: score matmuls run 3 k-tiles ahead
of the attn@V matmuls (so PE never waits on exp), the q/k projections
for later head-groups are interleaved into the attention stream (keeps
the PE HAM clock-gate at full speed), and DMA traffic is spread over
the sync/scalar/gpsimd queues so the lead-in is not serialized on one
engine. A burst of dummy matmuls on the first-arriving const tile
un-throttles the HAM clock gate (1.2 -> 2.4 GHz) during the load phase.
"""

import sys

try:
    import concourse.bass as bass  # noqa: F401
except Exception:
    sys.path.insert(0, "/opt/trn_rl_repo")

import ml_dtypes
import numpy as np

import concourse.bass as bass
import concourse.mybir as mybir
import concourse.tile as tile
from concourse import bacc
from concourse.bass_utils import run_bass_kernel_spmd

F32 = mybir.dt.float32
F32R = mybir.dt.float32r
BF16 = mybir.dt.bfloat16
AF = mybir.ActivationFunctionType
BF = ml_dtypes.bfloat16

B, S, E = 4, 1024, 1024
H, D = 16, 64
HG = 2              # head groups (cores per batch)
HPG = H // HG       # 8 heads per group
EG = HPG * D        # 512 embed cols per group
P = 128
ET = E // P         # 8 embed tiles
RT = S // P         # 8 row tiles
CT = EG // P        # 4 col tiles of the group's q/k
QCH = 512           # q-chunk (moving free dim; ISA max for fp32 PSUM out)
NQC = S // QCH      # 2 q chunks
KTQ = QCH // P      # 4 k-tiles per q chunk
SCALE = 1.0 / np.sqrt(D)


def _emit(nc, tc, with_bias):
    # inputs pre-packed on host into [128, *] layouts with long
    # contiguous rows so each loads as ONE descriptor-light DMA
    xT = nc.dram_tensor("xT", [P, ET * S], BF16, kind="ExternalInput")
    wq = nc.dram_tensor("wq", [P, ET * EG], BF16, kind="ExternalInput")
    wk = nc.dram_tensor("wk", [P, ET * EG], BF16, kind="ExternalInput")
    wv = nc.dram_tensor("wv", [P, ET * EG], BF16, kind="ExternalInput")
    wp = nc.dram_tensor("wp", [P, CT * E], BF16, kind="ExternalInput")
    # packed constants: cb = binary causal mask(128) | vones(8)  (bf16)
    cb = nc.dram_tensor("cb", [P, P + HPG], BF16, kind="ExternalInput")
    # bqk = bq(4) | bk(4)  (f32, per-partition bias)
    bqk = nc.dram_tensor("bqk", [P, 2 * CT], F32, kind="ExternalInput")
    # crow = ones(512) | bv(512) | bp(1024)  (f32 rows)
    crow = nc.dram_tensor("crow", [1, QCH + EG + E], F32,
                          kind="ExternalInput")
    out = nc.dram_tensor("out", [S, E], BF16, kind="ExternalOutput")

    with (
        tc.tile_pool(name="big", bufs=1) as p_big,
        tc.tile_pool(name="exs", bufs=6) as p_ex,
        tc.tile_pool(name="rc", bufs=4) as p_rc,
        tc.tile_pool(name="rcb", bufs=4) as p_rcb,
        tc.tile_pool(name="osb", bufs=4) as p_osb,
        tc.tile_pool(name="sm", bufs=1) as p_sm,
        tc.tile_pool(name="sc", bufs=3, space="PSUM") as p_sc,
        tc.tile_pool(name="qk", bufs=2, space="PSUM") as p_qk,
        tc.tile_pool(name="avp", bufs=3, space="PSUM") as p_av,
    ):
        # ---- constants: packed DMAs on the gpsimd queue ----
        cb_sb = p_sm.tile([P, P + HPG], BF16, tag="cb", name="cbt")
        nc.gpsimd.dma_start(cb_sb[:], cb[:])
        mask_sb = cb_sb[:, 0:P]
        vones_sb = cb_sb[:, P:P + HPG]
        bqk_sb = p_sm.tile([P, 2 * CT], F32, tag="bqk", name="bqkt")
        nc.gpsimd.dma_start(bqk_sb[:], bqk[:])
        bq_sb = bqk_sb[:, 0:CT]
        bk_sb = bqk_sb[:, CT:2 * CT]
        crow_sb = p_sm.tile([1, QCH + EG + E], F32, tag="crow", name="crowt")
        nc.gpsimd.dma_start(crow_sb[:], crow[:])
        ones_sb = crow_sb[:, 0:QCH].bitcast(F32R)
        bv_sb = crow_sb[:, QCH:QCH + EG].bitcast(F32R)
        bp_sb = crow_sb[:, QCH + EG:].bitcast(F32R)

        # ---- persistent sbuf tiles ----
        xt_b = p_big.tile([P, ET * S], BF16, tag="xtb", name="xtb")
        xt_t = [xt_b[:, et * S:(et + 1) * S] for et in range(ET)]
        wq_b = p_big.tile([P, ET * EG], BF16, tag="wqb", name="wqb")
        wq_t = [wq_b[:, et * EG:(et + 1) * EG] for et in range(ET)]
        wk_b = p_big.tile([P, ET * EG], BF16, tag="wkb", name="wkb")
        wk_t = [wk_b[:, et * EG:(et + 1) * EG] for et in range(ET)]
        wv_b = p_big.tile([P, ET * EG], BF16, tag="wvb", name="wvb")
        wv_t = [wv_b[:, et * EG:(et + 1) * EG] for et in range(ET)]
        wp_b = p_big.tile([P, CT * E], BF16, tag="wpb", name="wpb")
        wp_t = [wp_b[:, et * E:(et + 1) * E] for et in range(CT)]
        qT_t = [p_big.tile([P, S], BF16, tag=f"qt{ct}", name=f"qt{ct}")
                for ct in range(CT)]
        kT_t = [p_big.tile([P, S], BF16, tag=f"kt{ct}", name=f"kt{ct}")
                for ct in range(CT)]
        va_t = [p_big.tile([P, HPG * (D + 1)], BF16, tag=f"va{rt}",
                           name=f"va{rt}") for rt in range(RT)]
        ctx_t = [p_big.tile([P, S], BF16, tag=f"cx{ct}", name=f"cx{ct}")
                 for ct in range(CT)]

        # ---- input DMA: consumption-ordered trios ----
        # each et's (xt first-half, wq, wk) lands in parallel across the
        # three DMA-capable queues (sync/scalar/gpsimd), so the q0/k0
        # projections chase arrivals with no cross-tensor queuing delay;
        # xt second halves + wv follow, wp last
        engs = [nc.sync, nc.scalar, nc.gpsimd]
        for et in range(ET):
            engs[et % 3].dma_start(
                xt_t[et][:, 0:QCH], xT[:, et * S:et * S + QCH])
            engs[(et + 1) % 3].dma_start(
                wq_t[et], wq[:, et * EG:(et + 1) * EG])
            engs[(et + 2) % 3].dma_start(
                wk_t[et], wk[:, et * EG:(et + 1) * EG])
        for et in range(ET):
            engs[et % 3].dma_start(
                xt_t[et][:, QCH:S], xT[:, et * S + QCH:(et + 1) * S])
            engs[(et + 1) % 3].dma_start(
                wv_t[et], wv[:, et * EG:(et + 1) * EG])
        for et in range(CT):
            engs[et % 3].dma_start(wp_t[et], wp[:, et * E:(et + 1) * E])

        # ---- q/k projection chunk: qT/kT[ct][:, rc*QCH:+QCH] ----
        def emit_qk_chunk(dst, w_t, b_sb, ct, rc, engine, fill=0):
            ps = p_qk.tile([P, QCH], F32, tag="qk", name="qk")
            for et in range(ET):
                nc.tensor.matmul(
                    ps[:],
                    w_t[et][:, ct * P:(ct + 1) * P],
                    xt_t[et][:, rc * QCH:(rc + 1) * QCH],
                    start=(et == 0), stop=(et == ET - 1),
                )
                warm(fill)
            dst_ap = dst[ct][:, rc * QCH:(rc + 1) * QCH]
            if with_bias:
                nc.scalar.activation(
                    dst_ap, ps[:], AF.Identity, bias=b_sb[:, ct:ct + 1])
            elif engine == "act":
                nc.scalar.activation(dst_ap, ps[:], AF.Copy)
            else:
                nc.vector.tensor_copy(dst_ap, ps[:])

        # ---- v projection + augmented-va pack for one row tile ----
        def emit_v_rt(rt, fill=0):
            va3 = va_t[rt][:].rearrange("p (h d) -> p h d", h=HPG)
            nc.vector.tensor_copy(
                va3[:, :, D:D + 1],
                vones_sb.rearrange("p (h o) -> p h o", o=1))
            ps = p_qk.tile([P, QCH], F32, tag="qk", name="qk")
            for et in range(ET):
                nc.tensor.matmul(
                    ps[:, 0:EG],
                    xt_t[et][:, rt * P:(rt + 1) * P],
                    wv_t[et][:],
                    start=(et == 0),
                    stop=(et == ET - 1 and not with_bias),
                )
                warm(fill)
            if with_bias:
                nc.tensor.matmul(
                    ps[:, 0:EG], ones_sb[0:1, 0:P], bv_sb[0:1, :],
                    start=False, stop=True,
                )
            ps3 = ps[:, 0:EG].rearrange("p (h d) -> p h d", h=HPG)
            nc.vector.tensor_copy(va3[:, :, 0:D], ps3[:])

        # ---- HAM warmup: ~4us of continuous dummy matmuls on the
        # first-arriving (tiny) const tile un-throttles the PE clock
        # gate (1.2 -> 2.4 GHz) before the real lead runs. Results are
        # discarded: the first real matmul's start=True clears PSUM.
        avw = p_av.tile([D + 1, QCH], F32, tag="av", name="av")
        warm_first = [True]

        def warm(n):
            for _ in range(n):
                nc.tensor.matmul(
                    avw[0:D, 0:P], mask_sb[:, 0:D], mask_sb,
                    start=warm_first[0], stop=False, skip_group_check=True,
                )
                warm_first[0] = False

        warm(44)

        # ---- lead phase (minimal): unit (h0,qc0) only needs the rc0
        # halves of q/k ct0 plus va0-3; everything else is injected into
        # the attention stream (converts on DVE: the Act engine's queue
        # is busy issuing wq/wv DMAs in this window) ----
        emit_qk_chunk(qT_t, wq_t, bq_sb, 0, 0, "dve")
        emit_qk_chunk(kT_t, wk_t, bk_sb, 0, 0, "dve")
        for rt in range(4):
            emit_v_rt(rt)

        # remaining q/k chunks + v row tiles, injected mid-attention:
        # dense full-K PE work that keeps the HAM clock-gate warm while
        # the queued exps keep the Act engine busy. Each chunk must land
        # one unit before its first consumer.
        def qk_inj(dst, w_t, b_sb, ct, rc):
            return lambda: emit_qk_chunk(dst, w_t, b_sb, ct, rc, "dve")

        inj = {0: [qk_inj(qT_t, wq_t, bq_sb, 0, 1),
                   qk_inj(kT_t, wk_t, bk_sb, 0, 1),
                   lambda: emit_v_rt(4),
                   lambda: emit_v_rt(5)],
               1: [lambda: emit_v_rt(6),
                   lambda: emit_v_rt(7)]}
        u = 2
        for ct in range(1, CT):
            inj[u] = [qk_inj(qT_t, wq_t, bq_sb, ct, 0)]
            inj[u + 1] = [qk_inj(kT_t, wk_t, bk_sb, ct, 0)]
            inj[u + 2] = [qk_inj(qT_t, wq_t, bq_sb, ct, 1)]
            inj[u + 3] = [qk_inj(kT_t, wk_t, bk_sb, ct, 1)]
            u += 4

        # ---- attention ----
        for h in range(HPG):
            hp, hb = h // 2, (h % 2) * D
            va3s = [va_t[kt][:].rearrange("p (h d) -> p h d", h=HPG)[:, h, :]
                    for kt in range(RT)]
            for qc in range(NQC):
                n_kt = (qc + 1) * KTQ
                av = p_av.tile([D + 1, QCH], F32, tag="av", name="av")
                exs = {}

                def emit_sc(kt, qc=qc, exs=exs):
                    off = max(0, kt - qc * KTQ) * P
                    n = QCH - off
                    diag = (qc == 0) or (kt >= KTQ)
                    sc = p_sc.tile([P, QCH], F32, tag="sc", name="sc")
                    nc.tensor.matmul(
                        sc[:, 0:n],
                        kT_t[hp][hb:hb + D, kt * P:(kt + 1) * P],
                        qT_t[hp][hb:hb + D,
                                 qc * QCH + off:(qc + 1) * QCH],
                        start=True, stop=True,
                        tile_position=(hb, 0),
                    )
                    ex = p_ex.tile([P, QCH], BF16, tag="ex", name="ex")
                    nc.scalar.activation(ex[:, 0:n], sc[:, 0:n], AF.Exp)
                    if diag:
                        nc.vector.tensor_mul(
                            ex[:, 0:P], ex[:, 0:P], mask_sb)
                    exs[kt] = (ex, off, n)

                LOOK = 3
                for kt in range(min(LOOK, n_kt)):
                    emit_sc(kt)
                # inject projection chunks mid-unit: the queued exps
                # keep the Act engine busy while PE runs them
                for fn in inj.get(h * NQC + qc, ()):
                    fn()
                for kt in range(n_kt):
                    ex, off, n = exs.pop(kt)
                    nc.tensor.matmul(
                        av[:, off:QCH],
                        va3s[kt],
                        ex[:, 0:n],
                        start=(kt == 0), stop=(kt == n_kt - 1),
                    )
                    if kt + LOOK < n_kt:
                        emit_sc(kt + LOOK)

                # normalize: all off the PE stream
                # (reciprocal_approx_fast's bitwise seed misreads PSUM,
                # so stage the denominator row through SBUF first)
                dn_sb = p_rc.tile([1, QCH], F32, tag="dn", name="dn")
                nc.vector.tensor_copy(dn_sb[:], av[D:D + 1, :])
                rc_sb = p_rc.tile([1, QCH], F32, tag="rc", name="rc")
                nc.vector.reciprocal_approx_fast(rc_sb[:], dn_sb[:])
                rcb = p_rcb.tile([D, QCH], F32, tag="rcb", name="rcb")
                nc.gpsimd.partition_broadcast(rcb[:], rc_sb[:], channels=D)
                nc.vector.tensor_mul(
                    ctx_t[hp][hb:hb + D, qc * QCH:(qc + 1) * QCH],
                    av[0:D, :], rcb[:])

        # ---- output projection: partial = ctxT.T @ wp (+ bp) ----
        osb_eng = 0
        for rt in range(RT):
            for cc in range(E // QCH):
                ps = p_sc.tile([P, QCH], F32, tag="sc", name="sc")
                for et in range(CT):
                    nc.tensor.matmul(
                        ps[:],
                        ctx_t[et][:, rt * P:(rt + 1) * P],
                        wp_t[et][:, cc * QCH:(cc + 1) * QCH],
                        start=(et == 0),
                        stop=(et == CT - 1 and not with_bias),
                    )
                if with_bias:
                    nc.tensor.matmul(
                        ps[:], ones_sb[0:1, 0:P],
                        bp_sb[0:1, cc * QCH:(cc + 1) * QCH],
                        start=False, stop=True,
                    )
                osb = p_osb.tile([P, QCH], BF16, tag="osb", name="osb")
                if osb_eng == 0:
                    nc.vector.tensor_copy(osb[:], ps[:])
                else:
                    nc.scalar.activation(osb[:], ps[:], AF.Copy)
                osb_eng = (osb_eng + 1) % 2
                dma_eng = nc.sync if cc == 0 else nc.gpsimd
                dma_eng.dma_start(
                    out[rt * P:(rt + 1) * P, cc * QCH:(cc + 1) * QCH],
                    osb[:])


def build_nc(with_bias=False):
    nc = bacc.Bacc("TRN2", target_bir_lowering=False, debug=False)
    with tile.TileContext(nc) as tc, nc.allow_low_precision(
        reason="bf16 matmul pipeline; fp32 PSUM accumulate"
    ):
        _emit(nc, tc, with_bias)
    nc.compile()
    return nc


def make_in_maps(x, Wqkv, bqkv, Wproj, bproj):
    x = np.asarray(x, dtype=np.float32)
    Wqkv = np.asarray(Wqkv, dtype=np.float32)
    bqkv = np.asarray(bqkv, dtype=np.float32)
    Wproj = np.asarray(Wproj, dtype=np.float32)
    bproj = np.asarray(bproj, dtype=np.float32)
    keep = np.triu(np.ones((P, P), dtype=np.float32))  # [k, q]: k <= q
    cb = np.concatenate([
        keep,                                 # binary causal mask
        np.ones((P, HPG), dtype=np.float32),  # vones
    ], axis=1).astype(BF)
    in_maps = []
    for c in range(8):
        b, hg = c // 2, c % 2
        g = slice(hg * EG, (hg + 1) * EG)
        bqk = np.concatenate([
            (bqkv[0 * E:1 * E][g] * SCALE).reshape(CT, P).T,
            bqkv[1 * E:2 * E][g].reshape(CT, P).T], axis=1)
        crow = np.concatenate([
            np.ones(QCH, dtype=np.float32),
            bqkv[2 * E:3 * E][g],
            bproj if hg == 0 else np.zeros_like(bproj),
        ]).reshape(1, QCH + EG + E)
        def pack(a):
            # [n*128, m] -> [128, n*m] with row-major et-chunks
            n = a.shape[0] // P
            return np.ascontiguousarray(
                a.reshape(n, P, a.shape[1]).transpose(1, 0, 2)
                .reshape(P, n * a.shape[1])).astype(BF)
        in_maps.append({
            "xT": pack(x[b].T),
            "wq": pack(Wqkv[:, 0 * E:1 * E][:, g] * SCALE),
            "wk": pack(Wqkv[:, 1 * E:2 * E][:, g]),
            "wv": pack(Wqkv[:, 2 * E:3 * E][:, g]),
            "wp": pack(Wproj[g, :]),
            "cb": cb,
            "bqk": np.ascontiguousarray(bqk),
            "crow": np.ascontiguousarray(crow),
        })
    return in_maps


def kernel(x, Wqkv, bqkv, Wproj, bproj):
    with_bias = bool(
        np.any(np.asarray(bqkv)) or np.any(np.asarray(bproj)))
    nc = build_nc(with_bias)
    in_maps = make_in_maps(x, Wqkv, bqkv, Wproj, bproj)
    res = run_bass_kernel_spmd(nc, in_maps, list(range(8))).results
    out = np.zeros((B, S, E), dtype=np.float32)
    for c in range(8):
        out[c // 2] += res[c]["out"].astype(np.float32)
    return out


# revision 30
# speedup vs baseline: 1.0293x; 1.0150x over previous
"""Causal multi-head attention on 8 trn2 NeuronCores.

Sharding: core c -> (batch b = c//2, head-group hg = c%2).
Each head-group owns 8 of the 16 heads (512 of the 1024 embed dims after
the head split).

v3 layout (all matmul operands bf16, PSUM accumulation fp32):
  - qT, kT = (x[b] @ Wq_hg)^T, (x[b] @ Wk_hg)^T    [cols, rows] bf16
    (softmax 1/sqrt(d) scale folded into Wq on host)
  - v packed as va [rows, 8*(64+1)] bf16 with a ones column per head so
    the attn@V matmul also produces the softmax denominator (row 64).
  - scoresT [k, q] per (head, 512-q-chunk, 128-k-tile); exp -> bf16 on
    the Act engine; causal-diagonal tiles then have their first 128
    columns multiplied by a binary mask on the DVE (fast 2-byte mode).
  - normalize: denom row -> SBUF -> reciprocal_approx_fast (DVE) ->
    partition_broadcast (GPSIMD) -> one DVE multiply into ctxT bf16.
  - partial = ctxT.T @ Wproj_hg (+ bproj on hg==0 cores).
Host: out[b] = partial(b,0) + partial(b,1).

Emission order is software-pipelined: score matmuls run 3 k-tiles ahead
of the attn@V matmuls (so PE never waits on exp), the q/k projections
for later head-groups are interleaved into the attention stream (keeps
the PE HAM clock-gate at full speed), and DMA traffic is spread over
the sync/scalar/gpsimd queues so the lead-in is not serialized on one
engine. A burst of dummy matmuls on the first-arriving const tile
un-throttles the HAM clock gate (1.2 -> 2.4 GHz) during the load phase.
"""

import sys

try:
    import concourse.bass as bass  # noqa: F401
except Exception:
    sys.path.insert(0, "/opt/trn_rl_repo")

import ml_dtypes
import numpy as np

import concourse.bass as bass
import concourse.mybir as mybir
import concourse.tile as tile
from concourse import bacc
from concourse.bass_utils import run_bass_kernel_spmd

F32 = mybir.dt.float32
F32R = mybir.dt.float32r
BF16 = mybir.dt.bfloat16
AF = mybir.ActivationFunctionType
BF = ml_dtypes.bfloat16

B, S, E = 4, 1024, 1024
H, D = 16, 64
HG = 2              # head groups (cores per batch)
HPG = H // HG       # 8 heads per group
EG = HPG * D        # 512 embed cols per group
P = 128
ET = E // P         # 8 embed tiles
RT = S // P         # 8 row tiles
CT = EG // P        # 4 col tiles of the group's q/k
QCH = 512           # q-chunk (moving free dim; ISA max for fp32 PSUM out)
NQC = S // QCH      # 2 q chunks
KTQ = QCH // P      # 4 k-tiles per q chunk
SCALE = 1.0 / np.sqrt(D)


def _emit(nc, tc, with_bias):
    # inputs pre-packed on host into [128, *] layouts with long
    # contiguous rows so each loads as ONE descriptor-light DMA
    xT = nc.dram_tensor("xT", [P, ET * S], BF16, kind="ExternalInput")
    wq = nc.dram_tensor("wq", [P, ET * EG], BF16, kind="ExternalInput")
    wk = nc.dram_tensor("wk", [P, ET * EG], BF16, kind="ExternalInput")
    wv = nc.dram_tensor("wv", [P, ET * EG], BF16, kind="ExternalInput")
    wp = nc.dram_tensor("wp", [P, CT * E], BF16, kind="ExternalInput")
    # packed constants: cb = binary causal mask(128) | vones(8)  (bf16)
    cb = nc.dram_tensor("cb", [P, P + HPG], BF16, kind="ExternalInput")
    # bqk = bq(4) | bk(4)  (f32, per-partition bias)
    bqk = nc.dram_tensor("bqk", [P, 2 * CT], F32, kind="ExternalInput")
    # crow = ones(512) | bv(512) | bp(1024)  (f32 rows)
    crow = nc.dram_tensor("crow", [1, QCH + EG + E], F32,
                          kind="ExternalInput")
    out = nc.dram_tensor("out", [S, E], BF16, kind="ExternalOutput")

    with (
        tc.tile_pool(name="big", bufs=1) as p_big,
        tc.tile_pool(name="exs", bufs=6) as p_ex,
        tc.tile_pool(name="rc", bufs=4) as p_rc,
        tc.tile_pool(name="rcb", bufs=4) as p_rcb,
        tc.tile_pool(name="osb", bufs=4) as p_osb,
        tc.tile_pool(name="sm", bufs=1) as p_sm,
        tc.tile_pool(name="sc", bufs=3, space="PSUM") as p_sc,
        tc.tile_pool(name="qk", bufs=2, space="PSUM") as p_qk,
        tc.tile_pool(name="avp", bufs=3, space="PSUM") as p_av,
    ):
        # ---- constants: packed DMAs on the gpsimd queue ----
        cb_sb = p_sm.tile([P, P + HPG], BF16, tag="cb", name="cbt")
        nc.gpsimd.dma_start(cb_sb[:], cb[:])
        mask_sb = cb_sb[:, 0:P]
        vones_sb = cb_sb[:, P:P + HPG]
        bqk_sb = p_sm.tile([P, 2 * CT], F32, tag="bqk", name="bqkt")
        nc.gpsimd.dma_start(bqk_sb[:], bqk[:])
        bq_sb = bqk_sb[:, 0:CT]
        bk_sb = bqk_sb[:, CT:2 * CT]
        crow_sb = p_sm.tile([1, QCH + EG + E], F32, tag="crow", name="crowt")
        nc.gpsimd.dma_start(crow_sb[:], crow[:])
        ones_sb = crow_sb[:, 0:QCH].bitcast(F32R)
        bv_sb = crow_sb[:, QCH:QCH + EG].bitcast(F32R)
        bp_sb = crow_sb[:, QCH + EG:].bitcast(F32R)

        # ---- persistent sbuf tiles ----
        xt_b = p_big.tile([P, ET * S], BF16, tag="xtb", name="xtb")
        xt_t = [xt_b[:, et * S:(et + 1) * S] for et in range(ET)]
        wq_b = p_big.tile([P, ET * EG], BF16, tag="wqb", name="wqb")
        wq_t = [wq_b[:, et * EG:(et + 1) * EG] for et in range(ET)]
        wk_b = p_big.tile([P, ET * EG], BF16, tag="wkb", name="wkb")
        wk_t = [wk_b[:, et * EG:(et + 1) * EG] for et in range(ET)]
        wv_b = p_big.tile([P, ET * EG], BF16, tag="wvb", name="wvb")
        wv_t = [wv_b[:, et * EG:(et + 1) * EG] for et in range(ET)]
        wp_b = p_big.tile([P, CT * E], BF16, tag="wpb", name="wpb")
        wp_t = [wp_b[:, et * E:(et + 1) * E] for et in range(CT)]
        qT_t = [p_big.tile([P, S], BF16, tag=f"qt{ct}", name=f"qt{ct}")
                for ct in range(CT)]
        kT_t = [p_big.tile([P, S], BF16, tag=f"kt{ct}", name=f"kt{ct}")
                for ct in range(CT)]
        va_t = [p_big.tile([P, HPG * (D + 1)], BF16, tag=f"va{rt}",
                           name=f"va{rt}") for rt in range(RT)]
        ctx_t = [p_big.tile([P, S], BF16, tag=f"cx{ct}", name=f"cx{ct}")
                 for ct in range(CT)]

        # ---- input DMA: consumption-ordered trios ----
        # each et's (xt first-half, wq, wk) lands in parallel across the
        # three DMA-capable queues (sync/scalar/gpsimd), so the q0/k0
        # projections chase arrivals with no cross-tensor queuing delay;
        # xt second halves + wv follow, wp last
        engs = [nc.sync, nc.scalar, nc.gpsimd]
        for et in range(ET):
            engs[et % 3].dma_start(
                xt_t[et][:, 0:QCH], xT[:, et * S:et * S + QCH])
            engs[(et + 1) % 3].dma_start(
                wq_t[et], wq[:, et * EG:(et + 1) * EG])
            engs[(et + 2) % 3].dma_start(
                wk_t[et], wk[:, et * EG:(et + 1) * EG])
        for et in range(ET):
            engs[et % 3].dma_start(
                xt_t[et][:, QCH:S], xT[:, et * S + QCH:(et + 1) * S])
            engs[(et + 1) % 3].dma_start(
                wv_t[et], wv[:, et * EG:(et + 1) * EG])
        for et in range(CT):
            engs[et % 3].dma_start(wp_t[et], wp[:, et * E:(et + 1) * E])

        # ---- q/k projection chunk: qT/kT[ct][:, rc*QCH:+QCH] ----
        def emit_qk_chunk(dst, w_t, b_sb, ct, rc, engine, fill=0):
            ps = p_qk.tile([P, QCH], F32, tag="qk", name="qk")
            for et in range(ET):
                nc.tensor.matmul(
                    ps[:],
                    w_t[et][:, ct * P:(ct + 1) * P],
                    xt_t[et][:, rc * QCH:(rc + 1) * QCH],
                    start=(et == 0), stop=(et == ET - 1),
                )
                warm(fill)
            dst_ap = dst[ct][:, rc * QCH:(rc + 1) * QCH]
            if with_bias:
                nc.scalar.activation(
                    dst_ap, ps[:], AF.Identity, bias=b_sb[:, ct:ct + 1])
            elif engine == "act":
                nc.scalar.activation(dst_ap, ps[:], AF.Copy)
            else:
                nc.vector.tensor_copy(dst_ap, ps[:])

        # ---- v projection + augmented-va pack for one row tile ----
        def emit_v_rt(rt, fill=0):
            va3 = va_t[rt][:].rearrange("p (h d) -> p h d", h=HPG)
            nc.vector.tensor_copy(
                va3[:, :, D:D + 1],
                vones_sb.rearrange("p (h o) -> p h o", o=1))
            ps = p_qk.tile([P, QCH], F32, tag="qk", name="qk")
            for et in range(ET):
                nc.tensor.matmul(
                    ps[:, 0:EG],
                    xt_t[et][:, rt * P:(rt + 1) * P],
                    wv_t[et][:],
                    start=(et == 0),
                    stop=(et == ET - 1 and not with_bias),
                )
                warm(fill)
            if with_bias:
                nc.tensor.matmul(
                    ps[:, 0:EG], ones_sb[0:1, 0:P], bv_sb[0:1, :],
                    start=False, stop=True,
                )
            ps3 = ps[:, 0:EG].rearrange("p (h d) -> p h d", h=HPG)
            nc.vector.tensor_copy(va3[:, :, 0:D], ps3[:])

        # ---- HAM warmup: ~4us of continuous dummy matmuls on the
        # first-arriving (tiny) const tile un-throttles the PE clock
        # gate (1.2 -> 2.4 GHz) before the real lead runs. Results are
        # discarded: the first real matmul's start=True clears PSUM.
        avw = p_av.tile([D + 1, QCH], F32, tag="av", name="av")
        warm_first = [True]

        def warm(n):
            for _ in range(n):
                nc.tensor.matmul(
                    avw[0:D, 0:P], mask_sb[:, 0:D], mask_sb,
                    start=warm_first[0], stop=False, skip_group_check=True,
                )
                warm_first[0] = False

        warm(44)

        # ---- lead phase (minimal): unit (h0,qc0) only needs the rc0
        # halves of q/k ct0 plus va0-3; everything else is injected into
        # the attention stream (converts on DVE: the Act engine's queue
        # is busy issuing wq/wv DMAs in this window) ----
        emit_qk_chunk(qT_t, wq_t, bq_sb, 0, 0, "dve")
        emit_qk_chunk(kT_t, wk_t, bk_sb, 0, 0, "dve")
        for rt in range(4):
            emit_v_rt(rt)

        # remaining q/k chunks + v row tiles, injected mid-attention:
        # dense full-K PE work that keeps the HAM clock-gate warm while
        # the queued exps keep the Act engine busy. Each chunk must land
        # one unit before its first consumer.
        def qk_inj(dst, w_t, b_sb, ct, rc):
            return lambda: emit_qk_chunk(dst, w_t, b_sb, ct, rc, "dve")

        inj = {0: [qk_inj(qT_t, wq_t, bq_sb, 0, 1),
                   qk_inj(kT_t, wk_t, bk_sb, 0, 1),
                   lambda: emit_v_rt(4),
                   lambda: emit_v_rt(5)],
               1: [lambda: emit_v_rt(6),
                   lambda: emit_v_rt(7)]}
        u = 2
        for ct in range(1, CT):
            inj[u] = [qk_inj(qT_t, wq_t, bq_sb, ct, 0)]
            inj[u + 1] = [qk_inj(kT_t, wk_t, bk_sb, ct, 0)]
            inj[u + 2] = [qk_inj(qT_t, wq_t, bq_sb, ct, 1)]
            inj[u + 3] = [qk_inj(kT_t, wk_t, bk_sb, ct, 1)]
            u += 4

        # ---- attention ----
        for h in range(HPG):
            hp, hb = h // 2, (h % 2) * D
            va3s = [va_t[kt][:].rearrange("p (h d) -> p h d", h=HPG)[:, h, :]
                    for kt in range(RT)]
            for qc in range(NQC):
                n_kt = (qc + 1) * KTQ
                av = p_av.tile([D + 1, QCH], F32, tag="av", name="av")
                exs = {}

                def emit_sc(kt, qc=qc, exs=exs):
                    off = max(0, kt - qc * KTQ) * P
                    n = QCH - off
                    diag = (qc == 0) or (kt >= KTQ)
                    sc = p_sc.tile([P, QCH], F32, tag="sc", name="sc")
                    nc.tensor.matmul(
                        sc[:, 0:n],
                        kT_t[hp][hb:hb + D, kt * P:(kt + 1) * P],
                        qT_t[hp][hb:hb + D,
                                 qc * QCH + off:(qc + 1) * QCH],
                        start=True, stop=True,
                        tile_position=(hb, 0),
                    )
                    ex = p_ex.tile([P, QCH], BF16, tag="ex", name="ex")
                    nc.scalar.activation(ex[:, 0:n], sc[:, 0:n], AF.Exp)
                    if diag:
                        nc.vector.tensor_mul(
                            ex[:, 0:P], ex[:, 0:P], mask_sb)
                    exs[kt] = (ex, off, n)

                LOOK = 3
                for kt in range(min(LOOK, n_kt)):
                    emit_sc(kt)
                # inject projection chunks mid-unit: the queued exps
                # keep the Act engine busy while PE runs them
                for fn in inj.get(h * NQC + qc, ()):
                    fn()
                # tail units have no injections left and are Act-paced;
                # a short dummy-matmul burst keeps the HAM clock gate
                # from re-throttling the PE to 1.2 GHz
                if h * NQC + qc >= 12:
                    dps = p_qk.tile([P, QCH], F32, tag="qk", name="qk")
                    for r in range(4):
                        nc.tensor.matmul(
                            dps[:], wq_t[0][:, 0:P], xt_t[0][:, 0:QCH],
                            start=(r == 0), stop=False,
                            skip_group_check=True,
                        )
                for kt in range(n_kt):
                    ex, off, n = exs.pop(kt)
                    nc.tensor.matmul(
                        av[:, off:QCH],
                        va3s[kt],
                        ex[:, 0:n],
                        start=(kt == 0), stop=(kt == n_kt - 1),
                    )
                    if kt + LOOK < n_kt:
                        emit_sc(kt + LOOK)

                # normalize: all off the PE stream
                # (reciprocal_approx_fast's bitwise seed misreads PSUM,
                # so stage the denominator row through SBUF first)
                dn_sb = p_rc.tile([1, QCH], F32, tag="dn", name="dn")
                nc.vector.tensor_copy(dn_sb[:], av[D:D + 1, :])
                rc_sb = p_rc.tile([1, QCH], F32, tag="rc", name="rc")
                nc.vector.reciprocal_approx_fast(rc_sb[:], dn_sb[:])
                rcb = p_rcb.tile([D, QCH], F32, tag="rcb", name="rcb")
                nc.gpsimd.partition_broadcast(rcb[:], rc_sb[:], channels=D)
                nc.vector.tensor_mul(
                    ctx_t[hp][hb:hb + D, qc * QCH:(qc + 1) * QCH],
                    av[0:D, :], rcb[:])

        # ---- output projection: partial = ctxT.T @ wp (+ bp) ----
        osb_eng = 0
        for rt in range(RT):
            for cc in range(E // QCH):
                ps = p_sc.tile([P, QCH], F32, tag="sc", name="sc")
                for et in range(CT):
                    nc.tensor.matmul(
                        ps[:],
                        ctx_t[et][:, rt * P:(rt + 1) * P],
                        wp_t[et][:, cc * QCH:(cc + 1) * QCH],
                        start=(et == 0),
                        stop=(et == CT - 1 and not with_bias),
                    )
                if with_bias:
                    nc.tensor.matmul(
                        ps[:], ones_sb[0:1, 0:P],
                        bp_sb[0:1, cc * QCH:(cc + 1) * QCH],
                        start=False, stop=True,
                    )
                osb = p_osb.tile([P, QCH], BF16, tag="osb", name="osb")
                if osb_eng == 0:
                    nc.vector.tensor_copy(osb[:], ps[:])
                else:
                    nc.scalar.activation(osb[:], ps[:], AF.Copy)
                osb_eng = (osb_eng + 1) % 2
                dma_eng = nc.sync if cc == 0 else nc.gpsimd
                dma_eng.dma_start(
                    out[rt * P:(rt + 1) * P, cc * QCH:(cc + 1) * QCH],
                    osb[:])


def build_nc(with_bias=False):
    nc = bacc.Bacc("TRN2", target_bir_lowering=False, debug=False)
    with tile.TileContext(nc) as tc, nc.allow_low_precision(
        reason="bf16 matmul pipeline; fp32 PSUM accumulate"
    ):
        _emit(nc, tc, with_bias)
    nc.compile()
    return nc


def make_in_maps(x, Wqkv, bqkv, Wproj, bproj):
    x = np.asarray(x, dtype=np.float32)
    Wqkv = np.asarray(Wqkv, dtype=np.float32)
    bqkv = np.asarray(bqkv, dtype=np.float32)
    Wproj = np.asarray(Wproj, dtype=np.float32)
    bproj = np.asarray(bproj, dtype=np.float32)
    keep = np.triu(np.ones((P, P), dtype=np.float32))  # [k, q]: k <= q
    cb = np.concatenate([
        keep,                                 # binary causal mask
        np.ones((P, HPG), dtype=np.float32),  # vones
    ], axis=1).astype(BF)
    in_maps = []
    for c in range(8):
        b, hg = c // 2, c % 2
        g = slice(hg * EG, (hg + 1) * EG)
        bqk = np.concatenate([
            (bqkv[0 * E:1 * E][g] * SCALE).reshape(CT, P).T,
            bqkv[1 * E:2 * E][g].reshape(CT, P).T], axis=1)
        crow = np.concatenate([
            np.ones(QCH, dtype=np.float32),
            bqkv[2 * E:3 * E][g],
            bproj if hg == 0 else np.zeros_like(bproj),
        ]).reshape(1, QCH + EG + E)
        def pack(a):
            # [n*128, m] -> [128, n*m] with row-major et-chunks
            n = a.shape[0] // P
            return np.ascontiguousarray(
                a.reshape(n, P, a.shape[1]).transpose(1, 0, 2)
                .reshape(P, n * a.shape[1])).astype(BF)
        in_maps.append({
            "xT": pack(x[b].T),
            "wq": pack(Wqkv[:, 0 * E:1 * E][:, g] * SCALE),
            "wk": pack(Wqkv[:, 1 * E:2 * E][:, g]),
            "wv": pack(Wqkv[:, 2 * E:3 * E][:, g]),
            "wp": pack(Wproj[g, :]),
            "cb": cb,
            "bqk": np.ascontiguousarray(bqk),
            "crow": np.ascontiguousarray(crow),
        })
    return in_maps


def kernel(x, Wqkv, bqkv, Wproj, bproj):
    with_bias = bool(
        np.any(np.asarray(bqkv)) or np.any(np.asarray(bproj)))
    nc = build_nc(with_bias)
    in_maps = make_in_maps(x, Wqkv, bqkv, Wproj, bproj)
    res = run_bass_kernel_spmd(nc, in_maps, list(range(8))).results
    out = np.zeros((B, S, E), dtype=np.float32)
    for c in range(8):
        out[c // 2] += res[c]["out"].astype(np.float32)
    return out


# revision 31
# speedup vs baseline: 1.0338x; 1.0044x over previous
"""Causal multi-head attention on 8 trn2 NeuronCores.

Sharding: core c -> (batch b = c//2, head-group hg = c%2).
Each head-group owns 8 of the 16 heads (512 of the 1024 embed dims after
the head split).

v3 layout (all matmul operands bf16, PSUM accumulation fp32):
  - qT, kT = (x[b] @ Wq_hg)^T, (x[b] @ Wk_hg)^T    [cols, rows] bf16
    (softmax 1/sqrt(d) scale folded into Wq on host)
  - v packed as va [rows, 8*(64+1)] bf16 with a ones column per head so
    the attn@V matmul also produces the softmax denominator (row 64).
  - scoresT [k, q] per (head, 512-q-chunk, 128-k-tile); exp -> bf16 on
    the Act engine; causal-diagonal tiles then have their first 128
    columns multiplied by a binary mask on the DVE (fast 2-byte mode).
  - normalize: denom row -> SBUF -> reciprocal_approx_fast (DVE) ->
    partition_broadcast (GPSIMD) -> one DVE multiply into ctxT bf16.
  - partial = ctxT.T @ Wproj_hg (+ bproj on hg==0 cores).
Host: out[b] = partial(b,0) + partial(b,1).

Emission order is software-pipelined: score matmuls run 3 k-tiles ahead
of the attn@V matmuls (so PE never waits on exp), the q/k projections
for later head-groups are interleaved into the attention stream (keeps
the PE HAM clock-gate at full speed), and DMA traffic is spread over
the sync/scalar/gpsimd queues so the lead-in is not serialized on one
engine. A burst of dummy matmuls on the first-arriving const tile
un-throttles the HAM clock gate (1.2 -> 2.4 GHz) during the load phase.
"""

import sys

try:
    import concourse.bass as bass  # noqa: F401
except Exception:
    sys.path.insert(0, "/opt/trn_rl_repo")

import ml_dtypes
import numpy as np

import concourse.bass as bass
import concourse.mybir as mybir
import concourse.tile as tile
from concourse import bacc
from concourse.bass_utils import run_bass_kernel_spmd

F32 = mybir.dt.float32
F32R = mybir.dt.float32r
BF16 = mybir.dt.bfloat16
AF = mybir.ActivationFunctionType
BF = ml_dtypes.bfloat16

B, S, E = 4, 1024, 1024
H, D = 16, 64
HG = 2              # head groups (cores per batch)
HPG = H // HG       # 8 heads per group
EG = HPG * D        # 512 embed cols per group
P = 128
ET = E // P         # 8 embed tiles
RT = S // P         # 8 row tiles
CT = EG // P        # 4 col tiles of the group's q/k
QCH = 512           # q-chunk (moving free dim; ISA max for fp32 PSUM out)
NQC = S // QCH      # 2 q chunks
KTQ = QCH // P      # 4 k-tiles per q chunk
SCALE = 1.0 / np.sqrt(D)


def _emit(nc, tc, with_bias):
    # inputs pre-packed on host into [128, *] layouts with long
    # contiguous rows so each loads as ONE descriptor-light DMA
    xT = nc.dram_tensor("xT", [P, ET * S], BF16, kind="ExternalInput")
    wq = nc.dram_tensor("wq", [P, ET * EG], BF16, kind="ExternalInput")
    wk = nc.dram_tensor("wk", [P, ET * EG], BF16, kind="ExternalInput")
    wv = nc.dram_tensor("wv", [P, ET * EG], BF16, kind="ExternalInput")
    wp = nc.dram_tensor("wp", [P, CT * E], BF16, kind="ExternalInput")
    # packed constants: cb = binary causal mask(128) | vones(8)  (bf16)
    cb = nc.dram_tensor("cb", [P, P + HPG], BF16, kind="ExternalInput")
    # bqk = bq(4) | bk(4)  (f32, per-partition bias)
    bqk = nc.dram_tensor("bqk", [P, 2 * CT], F32, kind="ExternalInput")
    # crow = ones(512) | bv(512) | bp(1024)  (f32 rows)
    crow = nc.dram_tensor("crow", [1, QCH + EG + E], F32,
                          kind="ExternalInput")
    out = nc.dram_tensor("out", [S, E], BF16, kind="ExternalOutput")

    with (
        tc.tile_pool(name="big", bufs=1) as p_big,
        tc.tile_pool(name="exs", bufs=6) as p_ex,
        tc.tile_pool(name="rc", bufs=4) as p_rc,
        tc.tile_pool(name="rcb", bufs=4) as p_rcb,
        tc.tile_pool(name="osb", bufs=4) as p_osb,
        tc.tile_pool(name="sm", bufs=1) as p_sm,
        tc.tile_pool(name="sc", bufs=3, space="PSUM") as p_sc,
        tc.tile_pool(name="qk", bufs=2, space="PSUM") as p_qk,
        tc.tile_pool(name="avp", bufs=3, space="PSUM") as p_av,
    ):
        # ---- constants: packed DMAs on the gpsimd queue ----
        cb_sb = p_sm.tile([P, P + HPG], BF16, tag="cb", name="cbt")
        nc.gpsimd.dma_start(cb_sb[:], cb[:])
        mask_sb = cb_sb[:, 0:P]
        vones_sb = cb_sb[:, P:P + HPG]
        bqk_sb = p_sm.tile([P, 2 * CT], F32, tag="bqk", name="bqkt")
        nc.gpsimd.dma_start(bqk_sb[:], bqk[:])
        bq_sb = bqk_sb[:, 0:CT]
        bk_sb = bqk_sb[:, CT:2 * CT]
        crow_sb = p_sm.tile([1, QCH + EG + E], F32, tag="crow", name="crowt")
        nc.gpsimd.dma_start(crow_sb[:], crow[:])
        ones_sb = crow_sb[:, 0:QCH].bitcast(F32R)
        bv_sb = crow_sb[:, QCH:QCH + EG].bitcast(F32R)
        bp_sb = crow_sb[:, QCH + EG:].bitcast(F32R)

        # ---- persistent sbuf tiles ----
        xt_b = p_big.tile([P, ET * S], BF16, tag="xtb", name="xtb")
        xt_t = [xt_b[:, et * S:(et + 1) * S] for et in range(ET)]
        wq_b = p_big.tile([P, ET * EG], BF16, tag="wqb", name="wqb")
        wq_t = [wq_b[:, et * EG:(et + 1) * EG] for et in range(ET)]
        wk_b = p_big.tile([P, ET * EG], BF16, tag="wkb", name="wkb")
        wk_t = [wk_b[:, et * EG:(et + 1) * EG] for et in range(ET)]
        wv_b = p_big.tile([P, ET * EG], BF16, tag="wvb", name="wvb")
        wv_t = [wv_b[:, et * EG:(et + 1) * EG] for et in range(ET)]
        wp_b = p_big.tile([P, CT * E], BF16, tag="wpb", name="wpb")
        wp_t = [wp_b[:, et * E:(et + 1) * E] for et in range(CT)]
        qT_t = [p_big.tile([P, S], BF16, tag=f"qt{ct}", name=f"qt{ct}")
                for ct in range(CT)]
        kT_t = [p_big.tile([P, S], BF16, tag=f"kt{ct}", name=f"kt{ct}")
                for ct in range(CT)]
        va_t = [p_big.tile([P, HPG * (D + 1)], BF16, tag=f"va{rt}",
                           name=f"va{rt}") for rt in range(RT)]
        ctx_t = [p_big.tile([P, S], BF16, tag=f"cx{ct}", name=f"cx{ct}")
                 for ct in range(CT)]

        # ---- input DMA: consumption-ordered trios ----
        # each et's (xt first-half, wq, wk) lands in parallel across the
        # three DMA-capable queues (sync/scalar/gpsimd), so the q0/k0
        # projections chase arrivals with no cross-tensor queuing delay;
        # xt second halves + wv follow, wp last
        engs = [nc.sync, nc.scalar, nc.gpsimd]
        for et in range(ET):
            engs[et % 3].dma_start(
                xt_t[et][:, 0:QCH], xT[:, et * S:et * S + QCH])
            engs[(et + 1) % 3].dma_start(
                wq_t[et], wq[:, et * EG:(et + 1) * EG])
            engs[(et + 2) % 3].dma_start(
                wk_t[et], wk[:, et * EG:(et + 1) * EG])
        for et in range(ET):
            engs[et % 3].dma_start(
                xt_t[et][:, QCH:S], xT[:, et * S + QCH:(et + 1) * S])
            engs[(et + 1) % 3].dma_start(
                wv_t[et], wv[:, et * EG:(et + 1) * EG])
        for et in range(CT):
            engs[et % 3].dma_start(wp_t[et], wp[:, et * E:(et + 1) * E])

        # ---- q/k projection chunk: qT/kT[ct][:, rc*QCH:+QCH] ----
        def emit_qk_chunk(dst, w_t, b_sb, ct, rc, engine, fill=0):
            ps = p_qk.tile([P, QCH], F32, tag="qk", name="qk")
            for et in range(ET):
                nc.tensor.matmul(
                    ps[:],
                    w_t[et][:, ct * P:(ct + 1) * P],
                    xt_t[et][:, rc * QCH:(rc + 1) * QCH],
                    start=(et == 0), stop=(et == ET - 1),
                )
                warm(fill)
            dst_ap = dst[ct][:, rc * QCH:(rc + 1) * QCH]
            if with_bias:
                nc.scalar.activation(
                    dst_ap, ps[:], AF.Identity, bias=b_sb[:, ct:ct + 1])
            elif engine == "act":
                nc.scalar.activation(dst_ap, ps[:], AF.Copy)
            else:
                nc.vector.tensor_copy(dst_ap, ps[:])

        # ---- v projection + augmented-va pack for one row tile ----
        def emit_v_rt(rt, fill=0):
            va3 = va_t[rt][:].rearrange("p (h d) -> p h d", h=HPG)
            nc.vector.tensor_copy(
                va3[:, :, D:D + 1],
                vones_sb.rearrange("p (h o) -> p h o", o=1))
            ps = p_qk.tile([P, QCH], F32, tag="qk", name="qk")
            for et in range(ET):
                nc.tensor.matmul(
                    ps[:, 0:EG],
                    xt_t[et][:, rt * P:(rt + 1) * P],
                    wv_t[et][:],
                    start=(et == 0),
                    stop=(et == ET - 1 and not with_bias),
                )
                warm(fill)
            if with_bias:
                nc.tensor.matmul(
                    ps[:, 0:EG], ones_sb[0:1, 0:P], bv_sb[0:1, :],
                    start=False, stop=True,
                )
            ps3 = ps[:, 0:EG].rearrange("p (h d) -> p h d", h=HPG)
            nc.vector.tensor_copy(va3[:, :, 0:D], ps3[:])

        # ---- HAM warmup: ~4us of continuous dummy matmuls on the
        # first-arriving (tiny) const tile un-throttles the PE clock
        # gate (1.2 -> 2.4 GHz) before the real lead runs. Results are
        # discarded: the first real matmul's start=True clears PSUM.
        avw = p_av.tile([D + 1, QCH], F32, tag="av", name="av")
        warm_first = [True]

        def warm(n):
            for _ in range(n):
                nc.tensor.matmul(
                    avw[0:D, 0:P], mask_sb[:, 0:D], mask_sb,
                    start=warm_first[0], stop=False, skip_group_check=True,
                )
                warm_first[0] = False

        warm(44)

        # ---- lead phase (minimal): unit (h0,qc0) only needs the rc0
        # halves of q/k ct0 plus va0-3; everything else is injected into
        # the attention stream (converts on DVE: the Act engine's queue
        # is busy issuing wq/wv DMAs in this window) ----
        emit_qk_chunk(qT_t, wq_t, bq_sb, 0, 0, "dve", fill=1)
        emit_qk_chunk(kT_t, wk_t, bk_sb, 0, 0, "dve", fill=1)
        for rt in range(4):
            emit_v_rt(rt)

        # remaining q/k chunks + v row tiles, injected mid-attention:
        # dense full-K PE work that keeps the HAM clock-gate warm while
        # the queued exps keep the Act engine busy. Each chunk must land
        # one unit before its first consumer.
        def qk_inj(dst, w_t, b_sb, ct, rc):
            return lambda: emit_qk_chunk(dst, w_t, b_sb, ct, rc, "dve")

        inj = {0: [qk_inj(qT_t, wq_t, bq_sb, 0, 1),
                   qk_inj(kT_t, wk_t, bk_sb, 0, 1),
                   lambda: emit_v_rt(4),
                   lambda: emit_v_rt(5)],
               1: [lambda: emit_v_rt(6),
                   lambda: emit_v_rt(7)]}
        u = 2
        for ct in range(1, CT):
            inj[u] = [qk_inj(qT_t, wq_t, bq_sb, ct, 0)]
            inj[u + 1] = [qk_inj(kT_t, wk_t, bk_sb, ct, 0)]
            inj[u + 2] = [qk_inj(qT_t, wq_t, bq_sb, ct, 1)]
            inj[u + 3] = [qk_inj(kT_t, wk_t, bk_sb, ct, 1)]
            u += 4

        # ---- attention ----
        for h in range(HPG):
            hp, hb = h // 2, (h % 2) * D
            va3s = [va_t[kt][:].rearrange("p (h d) -> p h d", h=HPG)[:, h, :]
                    for kt in range(RT)]
            for qc in range(NQC):
                n_kt = (qc + 1) * KTQ
                av = p_av.tile([D + 1, QCH], F32, tag="av", name="av")
                exs = {}

                def emit_sc(kt, qc=qc, exs=exs):
                    off = max(0, kt - qc * KTQ) * P
                    n = QCH - off
                    diag = (qc == 0) or (kt >= KTQ)
                    sc = p_sc.tile([P, QCH], F32, tag="sc", name="sc")
                    nc.tensor.matmul(
                        sc[:, 0:n],
                        kT_t[hp][hb:hb + D, kt * P:(kt + 1) * P],
                        qT_t[hp][hb:hb + D,
                                 qc * QCH + off:(qc + 1) * QCH],
                        start=True, stop=True,
                        tile_position=(hb, 0),
                    )
                    ex = p_ex.tile([P, QCH], BF16, tag="ex", name="ex")
                    nc.scalar.activation(ex[:, 0:n], sc[:, 0:n], AF.Exp)
                    if diag:
                        nc.vector.tensor_mul(
                            ex[:, 0:P], ex[:, 0:P], mask_sb)
                    exs[kt] = (ex, off, n)

                LOOK = 3
                for kt in range(min(LOOK, n_kt)):
                    emit_sc(kt)
                # inject projection chunks mid-unit: the queued exps
                # keep the Act engine busy while PE runs them
                for fn in inj.get(h * NQC + qc, ()):
                    fn()
                # tail units have no injections left and are Act-paced;
                # a short dummy-matmul burst keeps the HAM clock gate
                # from re-throttling the PE to 1.2 GHz
                if h * NQC + qc >= 12:
                    dps = p_qk.tile([P, QCH], F32, tag="qk", name="qk")
                    for r in range(4):
                        nc.tensor.matmul(
                            dps[:], wq_t[0][:, 0:P], xt_t[0][:, 0:QCH],
                            start=(r == 0), stop=False,
                            skip_group_check=True,
                        )
                for kt in range(n_kt):
                    ex, off, n = exs.pop(kt)
                    nc.tensor.matmul(
                        av[:, off:QCH],
                        va3s[kt],
                        ex[:, 0:n],
                        start=(kt == 0), stop=(kt == n_kt - 1),
                    )
                    if kt + LOOK < n_kt:
                        emit_sc(kt + LOOK)

                # normalize: all off the PE stream
                # (reciprocal_approx_fast's bitwise seed misreads PSUM,
                # so stage the denominator row through SBUF first)
                dn_sb = p_rc.tile([1, QCH], F32, tag="dn", name="dn")
                nc.vector.tensor_copy(dn_sb[:], av[D:D + 1, :])
                rc_sb = p_rc.tile([1, QCH], F32, tag="rc", name="rc")
                nc.vector.reciprocal_approx_fast(rc_sb[:], dn_sb[:])
                rcb = p_rcb.tile([D, QCH], F32, tag="rcb", name="rcb")
                nc.gpsimd.partition_broadcast(rcb[:], rc_sb[:], channels=D)
                nc.vector.tensor_mul(
                    ctx_t[hp][hb:hb + D, qc * QCH:(qc + 1) * QCH],
                    av[0:D, :], rcb[:])

        # ---- output projection: partial = ctxT.T @ wp (+ bp) ----
        osb_eng = 0
        for rt in range(RT):
            for cc in range(E // QCH):
                ps = p_sc.tile([P, QCH], F32, tag="sc", name="sc")
                for et in range(CT):
                    nc.tensor.matmul(
                        ps[:],
                        ctx_t[et][:, rt * P:(rt + 1) * P],
                        wp_t[et][:, cc * QCH:(cc + 1) * QCH],
                        start=(et == 0),
                        stop=(et == CT - 1 and not with_bias),
                    )
                if with_bias:
                    nc.tensor.matmul(
                        ps[:], ones_sb[0:1, 0:P],
                        bp_sb[0:1, cc * QCH:(cc + 1) * QCH],
                        start=False, stop=True,
                    )
                osb = p_osb.tile([P, QCH], BF16, tag="osb", name="osb")
                if osb_eng == 0:
                    nc.vector.tensor_copy(osb[:], ps[:])
                else:
                    nc.scalar.activation(osb[:], ps[:], AF.Copy)
                osb_eng = (osb_eng + 1) % 2
                dma_eng = nc.sync if cc == 0 else nc.gpsimd
                dma_eng.dma_start(
                    out[rt * P:(rt + 1) * P, cc * QCH:(cc + 1) * QCH],
                    osb[:])


def build_nc(with_bias=False):
    nc = bacc.Bacc("TRN2", target_bir_lowering=False, debug=False)
    with tile.TileContext(nc) as tc, nc.allow_low_precision(
        reason="bf16 matmul pipeline; fp32 PSUM accumulate"
    ):
        _emit(nc, tc, with_bias)
    nc.compile()
    return nc


def make_in_maps(x, Wqkv, bqkv, Wproj, bproj):
    x = np.asarray(x, dtype=np.float32)
    Wqkv = np.asarray(Wqkv, dtype=np.float32)
    bqkv = np.asarray(bqkv, dtype=np.float32)
    Wproj = np.asarray(Wproj, dtype=np.float32)
    bproj = np.asarray(bproj, dtype=np.float32)
    keep = np.triu(np.ones((P, P), dtype=np.float32))  # [k, q]: k <= q
    cb = np.concatenate([
        keep,                                 # binary causal mask
        np.ones((P, HPG), dtype=np.float32),  # vones
    ], axis=1).astype(BF)
    in_maps = []
    for c in range(8):
        b, hg = c // 2, c % 2
        g = slice(hg * EG, (hg + 1) * EG)
        bqk = np.concatenate([
            (bqkv[0 * E:1 * E][g] * SCALE).reshape(CT, P).T,
            bqkv[1 * E:2 * E][g].reshape(CT, P).T], axis=1)
        crow = np.concatenate([
            np.ones(QCH, dtype=np.float32),
            bqkv[2 * E:3 * E][g],
            bproj if hg == 0 else np.zeros_like(bproj),
        ]).reshape(1, QCH + EG + E)
        def pack(a):
            # [n*128, m] -> [128, n*m] with row-major et-chunks
            n = a.shape[0] // P
            return np.ascontiguousarray(
                a.reshape(n, P, a.shape[1]).transpose(1, 0, 2)
                .reshape(P, n * a.shape[1])).astype(BF)
        in_maps.append({
            "xT": pack(x[b].T),
            "wq": pack(Wqkv[:, 0 * E:1 * E][:, g] * SCALE),
            "wk": pack(Wqkv[:, 1 * E:2 * E][:, g]),
            "wv": pack(Wqkv[:, 2 * E:3 * E][:, g]),
            "wp": pack(Wproj[g, :]),
            "cb": cb,
            "bqk": np.ascontiguousarray(bqk),
            "crow": np.ascontiguousarray(crow),
        })
    return in_maps


def kernel(x, Wqkv, bqkv, Wproj, bproj):
    with_bias = bool(
        np.any(np.asarray(bqkv)) or np.any(np.asarray(bproj)))
    nc = build_nc(with_bias)
    in_maps = make_in_maps(x, Wqkv, bqkv, Wproj, bproj)
    res = run_bass_kernel_spmd(nc, in_maps, list(range(8))).results
    out = np.zeros((B, S, E), dtype=np.float32)
    for c in range(8):
        out[c // 2] += res[c]["out"].astype(np.float32)
    return out


# revision 32
# speedup vs baseline: 1.0359x; 1.0020x over previous
"""Causal multi-head attention on 8 trn2 NeuronCores.

Sharding: core c -> (batch b = c//2, head-group hg = c%2).
Each head-group owns 8 of the 16 heads (512 of the 1024 embed dims after
the head split).

v3 layout (all matmul operands bf16, PSUM accumulation fp32):
  - qT, kT = (x[b] @ Wq_hg)^T, (x[b] @ Wk_hg)^T    [cols, rows] bf16
    (softmax 1/sqrt(d) scale folded into Wq on host)
  - v packed as va [rows, 8*(64+1)] bf16 with a ones column per head so
    the attn@V matmul also produces the softmax denominator (row 64).
  - scoresT [k, q] per (head, 512-q-chunk, 128-k-tile); exp -> bf16 on
    the Act engine; causal-diagonal tiles then have their first 128
    columns multiplied by a binary mask on the DVE (fast 2-byte mode).
  - normalize: denom row -> SBUF -> reciprocal_approx_fast (DVE) ->
    partition_broadcast (GPSIMD) -> one DVE multiply into ctxT bf16.
  - partial = ctxT.T @ Wproj_hg (+ bproj on hg==0 cores).
Host: out[b] = partial(b,0) + partial(b,1).

Emission order is software-pipelined: score matmuls run 3 k-tiles ahead
of the attn@V matmuls (so PE never waits on exp), the q/k projections
for later head-groups are interleaved into the attention stream (keeps
the PE HAM clock-gate at full speed), and DMA traffic is spread over
the sync/scalar/gpsimd queues so the lead-in is not serialized on one
engine. A burst of dummy matmuls on the first-arriving const tile
un-throttles the HAM clock gate (1.2 -> 2.4 GHz) during the load phase.
"""

import sys

try:
    import concourse.bass as bass  # noqa: F401
except Exception:
    sys.path.insert(0, "/opt/trn_rl_repo")

import ml_dtypes
import numpy as np

import concourse.bass as bass
import concourse.mybir as mybir
import concourse.tile as tile
from concourse import bacc
from concourse.bass_utils import run_bass_kernel_spmd

F32 = mybir.dt.float32
F32R = mybir.dt.float32r
BF16 = mybir.dt.bfloat16
AF = mybir.ActivationFunctionType
BF = ml_dtypes.bfloat16

B, S, E = 4, 1024, 1024
H, D = 16, 64
HG = 2              # head groups (cores per batch)
HPG = H // HG       # 8 heads per group
EG = HPG * D        # 512 embed cols per group
P = 128
ET = E // P         # 8 embed tiles
RT = S // P         # 8 row tiles
CT = EG // P        # 4 col tiles of the group's q/k
QCH = 512           # q-chunk (moving free dim; ISA max for fp32 PSUM out)
NQC = S // QCH      # 2 q chunks
KTQ = QCH // P      # 4 k-tiles per q chunk
SCALE = 1.0 / np.sqrt(D)


def _emit(nc, tc, with_bias):
    # inputs pre-packed on host into [128, *] layouts with long
    # contiguous rows so each loads as ONE descriptor-light DMA
    xT = nc.dram_tensor("xT", [P, ET * S], BF16, kind="ExternalInput")
    wq = nc.dram_tensor("wq", [P, ET * EG], BF16, kind="ExternalInput")
    wk = nc.dram_tensor("wk", [P, ET * EG], BF16, kind="ExternalInput")
    wv = nc.dram_tensor("wv", [P, ET * EG], BF16, kind="ExternalInput")
    wp = nc.dram_tensor("wp", [P, CT * E], BF16, kind="ExternalInput")
    # packed constants: cb = binary causal mask(128) | vones(8)  (bf16)
    cb = nc.dram_tensor("cb", [P, P + HPG], BF16, kind="ExternalInput")
    # bqk = bq(4) | bk(4)  (f32, per-partition bias)
    bqk = nc.dram_tensor("bqk", [P, 2 * CT], F32, kind="ExternalInput")
    # crow = ones(512) | bv(512) | bp(1024)  (f32 rows)
    crow = nc.dram_tensor("crow", [1, QCH + EG + E], F32,
                          kind="ExternalInput")
    out = nc.dram_tensor("out", [S, E], BF16, kind="ExternalOutput")

    with (
        tc.tile_pool(name="big", bufs=1) as p_big,
        tc.tile_pool(name="exs", bufs=6) as p_ex,
        tc.tile_pool(name="rc", bufs=4) as p_rc,
        tc.tile_pool(name="rcb", bufs=4) as p_rcb,
        tc.tile_pool(name="osb", bufs=4) as p_osb,
        tc.tile_pool(name="sm", bufs=1) as p_sm,
        tc.tile_pool(name="sc", bufs=3, space="PSUM") as p_sc,
        tc.tile_pool(name="qk", bufs=2, space="PSUM") as p_qk,
        tc.tile_pool(name="avp", bufs=3, space="PSUM") as p_av,
    ):
        # ---- constants: packed DMAs on the gpsimd queue ----
        cb_sb = p_sm.tile([P, P + HPG], BF16, tag="cb", name="cbt")
        nc.gpsimd.dma_start(cb_sb[:], cb[:])
        mask_sb = cb_sb[:, 0:P]
        vones_sb = cb_sb[:, P:P + HPG]
        bqk_sb = p_sm.tile([P, 2 * CT], F32, tag="bqk", name="bqkt")
        nc.gpsimd.dma_start(bqk_sb[:], bqk[:])
        bq_sb = bqk_sb[:, 0:CT]
        bk_sb = bqk_sb[:, CT:2 * CT]
        crow_sb = p_sm.tile([1, QCH + EG + E], F32, tag="crow", name="crowt")
        nc.gpsimd.dma_start(crow_sb[:], crow[:])
        ones_sb = crow_sb[:, 0:QCH].bitcast(F32R)
        bv_sb = crow_sb[:, QCH:QCH + EG].bitcast(F32R)
        bp_sb = crow_sb[:, QCH + EG:].bitcast(F32R)

        # ---- persistent sbuf tiles ----
        xt_b = p_big.tile([P, ET * S], BF16, tag="xtb", name="xtb")
        xt_t = [xt_b[:, et * S:(et + 1) * S] for et in range(ET)]
        wq_b = p_big.tile([P, ET * EG], BF16, tag="wqb", name="wqb")
        wq_t = [wq_b[:, et * EG:(et + 1) * EG] for et in range(ET)]
        wk_b = p_big.tile([P, ET * EG], BF16, tag="wkb", name="wkb")
        wk_t = [wk_b[:, et * EG:(et + 1) * EG] for et in range(ET)]
        wv_b = p_big.tile([P, ET * EG], BF16, tag="wvb", name="wvb")
        wv_t = [wv_b[:, et * EG:(et + 1) * EG] for et in range(ET)]
        wp_b = p_big.tile([P, CT * E], BF16, tag="wpb", name="wpb")
        wp_t = [wp_b[:, et * E:(et + 1) * E] for et in range(CT)]
        qT_t = [p_big.tile([P, S], BF16, tag=f"qt{ct}", name=f"qt{ct}")
                for ct in range(CT)]
        kT_t = [p_big.tile([P, S], BF16, tag=f"kt{ct}", name=f"kt{ct}")
                for ct in range(CT)]
        va_t = [p_big.tile([P, HPG * (D + 1)], BF16, tag=f"va{rt}",
                           name=f"va{rt}") for rt in range(RT)]
        ctx_t = [p_big.tile([P, S], BF16, tag=f"cx{ct}", name=f"cx{ct}")
                 for ct in range(CT)]

        # ---- input DMA: consumption-ordered trios ----
        # each et's (xt first-half, wq, wk) lands in parallel across the
        # three DMA-capable queues (sync/scalar/gpsimd), so the q0/k0
        # projections chase arrivals with no cross-tensor queuing delay;
        # xt second halves + wv follow, wp last
        engs = [nc.sync, nc.scalar, nc.gpsimd]
        for et in range(ET):
            engs[et % 3].dma_start(
                xt_t[et][:, 0:QCH], xT[:, et * S:et * S + QCH])
            engs[(et + 1) % 3].dma_start(
                wq_t[et], wq[:, et * EG:(et + 1) * EG])
            engs[(et + 2) % 3].dma_start(
                wk_t[et], wk[:, et * EG:(et + 1) * EG])
        for et in range(ET):
            engs[et % 3].dma_start(
                xt_t[et][:, QCH:S], xT[:, et * S + QCH:(et + 1) * S])
            engs[(et + 1) % 3].dma_start(
                wv_t[et], wv[:, et * EG:(et + 1) * EG])
        for et in range(CT):
            engs[et % 3].dma_start(wp_t[et], wp[:, et * E:(et + 1) * E])

        # ---- q/k projection chunk: qT/kT[ct][:, rc*QCH:+QCH] ----
        def emit_qk_chunk(dst, w_t, b_sb, ct, rc, engine, fill=0):
            ps = p_qk.tile([P, QCH], F32, tag="qk", name="qk")
            for et in range(ET):
                nc.tensor.matmul(
                    ps[:],
                    w_t[et][:, ct * P:(ct + 1) * P],
                    xt_t[et][:, rc * QCH:(rc + 1) * QCH],
                    start=(et == 0), stop=(et == ET - 1),
                )
                warm(fill)
            dst_ap = dst[ct][:, rc * QCH:(rc + 1) * QCH]
            if with_bias:
                nc.scalar.activation(
                    dst_ap, ps[:], AF.Identity, bias=b_sb[:, ct:ct + 1])
            elif engine == "act":
                nc.scalar.activation(dst_ap, ps[:], AF.Copy)
            else:
                nc.vector.tensor_copy(dst_ap, ps[:])

        # ---- v projection + augmented-va pack for one row tile ----
        def emit_v_rt(rt, fill=0):
            va3 = va_t[rt][:].rearrange("p (h d) -> p h d", h=HPG)
            nc.vector.tensor_copy(
                va3[:, :, D:D + 1],
                vones_sb.rearrange("p (h o) -> p h o", o=1))
            ps = p_qk.tile([P, QCH], F32, tag="qk", name="qk")
            for et in range(ET):
                nc.tensor.matmul(
                    ps[:, 0:EG],
                    xt_t[et][:, rt * P:(rt + 1) * P],
                    wv_t[et][:],
                    start=(et == 0),
                    stop=(et == ET - 1 and not with_bias),
                )
                warm(fill)
            if with_bias:
                nc.tensor.matmul(
                    ps[:, 0:EG], ones_sb[0:1, 0:P], bv_sb[0:1, :],
                    start=False, stop=True,
                )
            ps3 = ps[:, 0:EG].rearrange("p (h d) -> p h d", h=HPG)
            nc.vector.tensor_copy(va3[:, :, 0:D], ps3[:])

        # ---- HAM warmup: ~4us of continuous dummy matmuls on the
        # first-arriving (tiny) const tile un-throttles the PE clock
        # gate (1.2 -> 2.4 GHz) before the real lead runs. Results are
        # discarded: the first real matmul's start=True clears PSUM.
        avw = p_av.tile([D + 1, QCH], F32, tag="av", name="av")
        warm_first = [True]

        def warm(n):
            for _ in range(n):
                nc.tensor.matmul(
                    avw[0:D, 0:P], mask_sb[:, 0:D], mask_sb,
                    start=warm_first[0], stop=False, skip_group_check=True,
                )
                warm_first[0] = False

        warm(44)

        # ---- lead phase (minimal): unit (h0,qc0) only needs the rc0
        # halves of q/k ct0 plus va0-3; everything else is injected into
        # the attention stream (converts on DVE: the Act engine's queue
        # is busy issuing wq/wv DMAs in this window) ----
        emit_qk_chunk(qT_t, wq_t, bq_sb, 0, 0, "dve", fill=1)
        emit_qk_chunk(kT_t, wk_t, bk_sb, 0, 0, "dve", fill=1)
        for rt in range(4):
            emit_v_rt(rt, fill=1 if rt < 2 else 0)

        # remaining q/k chunks + v row tiles, injected mid-attention:
        # dense full-K PE work that keeps the HAM clock-gate warm while
        # the queued exps keep the Act engine busy. Each chunk must land
        # one unit before its first consumer.
        def qk_inj(dst, w_t, b_sb, ct, rc):
            return lambda: emit_qk_chunk(dst, w_t, b_sb, ct, rc, "dve")

        inj = {0: [qk_inj(qT_t, wq_t, bq_sb, 0, 1),
                   qk_inj(kT_t, wk_t, bk_sb, 0, 1),
                   lambda: emit_v_rt(4),
                   lambda: emit_v_rt(5)],
               1: [lambda: emit_v_rt(6),
                   lambda: emit_v_rt(7)]}
        u = 2
        for ct in range(1, CT):
            inj[u] = [qk_inj(qT_t, wq_t, bq_sb, ct, 0)]
            inj[u + 1] = [qk_inj(kT_t, wk_t, bk_sb, ct, 0)]
            inj[u + 2] = [qk_inj(qT_t, wq_t, bq_sb, ct, 1)]
            inj[u + 3] = [qk_inj(kT_t, wk_t, bk_sb, ct, 1)]
            u += 4

        # ---- attention ----
        for h in range(HPG):
            hp, hb = h // 2, (h % 2) * D
            va3s = [va_t[kt][:].rearrange("p (h d) -> p h d", h=HPG)[:, h, :]
                    for kt in range(RT)]
            for qc in range(NQC):
                n_kt = (qc + 1) * KTQ
                av = p_av.tile([D + 1, QCH], F32, tag="av", name="av")
                exs = {}

                def emit_sc(kt, qc=qc, exs=exs):
                    off = max(0, kt - qc * KTQ) * P
                    n = QCH - off
                    diag = (qc == 0) or (kt >= KTQ)
                    sc = p_sc.tile([P, QCH], F32, tag="sc", name="sc")
                    nc.tensor.matmul(
                        sc[:, 0:n],
                        kT_t[hp][hb:hb + D, kt * P:(kt + 1) * P],
                        qT_t[hp][hb:hb + D,
                                 qc * QCH + off:(qc + 1) * QCH],
                        start=True, stop=True,
                        tile_position=(hb, 0),
                    )
                    ex = p_ex.tile([P, QCH], BF16, tag="ex", name="ex")
                    nc.scalar.activation(ex[:, 0:n], sc[:, 0:n], AF.Exp)
                    if diag:
                        nc.vector.tensor_mul(
                            ex[:, 0:P], ex[:, 0:P], mask_sb)
                    exs[kt] = (ex, off, n)

                LOOK = 3
                for kt in range(min(LOOK, n_kt)):
                    emit_sc(kt)
                # inject projection chunks mid-unit: the queued exps
                # keep the Act engine busy while PE runs them
                for fn in inj.get(h * NQC + qc, ()):
                    fn()
                # tail units have no injections left and are Act-paced;
                # a short dummy-matmul burst keeps the HAM clock gate
                # from re-throttling the PE to 1.2 GHz
                if h * NQC + qc >= 12:
                    dps = p_qk.tile([P, QCH], F32, tag="qk", name="qk")
                    for r in range(4):
                        nc.tensor.matmul(
                            dps[:], wq_t[0][:, 0:P], xt_t[0][:, 0:QCH],
                            start=(r == 0), stop=False,
                            skip_group_check=True,
                        )
                for kt in range(n_kt):
                    ex, off, n = exs.pop(kt)
                    nc.tensor.matmul(
                        av[:, off:QCH],
                        va3s[kt],
                        ex[:, 0:n],
                        start=(kt == 0), stop=(kt == n_kt - 1),
                    )
                    if kt + LOOK < n_kt:
                        emit_sc(kt + LOOK)

                # normalize: all off the PE stream
                # (reciprocal_approx_fast's bitwise seed misreads PSUM,
                # so stage the denominator row through SBUF first)
                dn_sb = p_rc.tile([1, QCH], F32, tag="dn", name="dn")
                nc.vector.tensor_copy(dn_sb[:], av[D:D + 1, :])
                rc_sb = p_rc.tile([1, QCH], F32, tag="rc", name="rc")
                nc.vector.reciprocal_approx_fast(rc_sb[:], dn_sb[:])
                rcb = p_rcb.tile([D, QCH], F32, tag="rcb", name="rcb")
                nc.gpsimd.partition_broadcast(rcb[:], rc_sb[:], channels=D)
                nc.vector.tensor_mul(
                    ctx_t[hp][hb:hb + D, qc * QCH:(qc + 1) * QCH],
                    av[0:D, :], rcb[:])

        # ---- output projection: partial = ctxT.T @ wp (+ bp) ----
        osb_eng = 0
        for rt in range(RT):
            for cc in range(E // QCH):
                ps = p_sc.tile([P, QCH], F32, tag="sc", name="sc")
                for et in range(CT):
                    nc.tensor.matmul(
                        ps[:],
                        ctx_t[et][:, rt * P:(rt + 1) * P],
                        wp_t[et][:, cc * QCH:(cc + 1) * QCH],
                        start=(et == 0),
                        stop=(et == CT - 1 and not with_bias),
                    )
                if with_bias:
                    nc.tensor.matmul(
                        ps[:], ones_sb[0:1, 0:P],
                        bp_sb[0:1, cc * QCH:(cc + 1) * QCH],
                        start=False, stop=True,
                    )
                osb = p_osb.tile([P, QCH], BF16, tag="osb", name="osb")
                if osb_eng == 0:
                    nc.vector.tensor_copy(osb[:], ps[:])
                else:
                    nc.scalar.activation(osb[:], ps[:], AF.Copy)
                osb_eng = (osb_eng + 1) % 2
                dma_eng = nc.sync if cc == 0 else nc.gpsimd
                dma_eng.dma_start(
                    out[rt * P:(rt + 1) * P, cc * QCH:(cc + 1) * QCH],
                    osb[:])


def build_nc(with_bias=False):
    nc = bacc.Bacc("TRN2", target_bir_lowering=False, debug=False)
    with tile.TileContext(nc) as tc, nc.allow_low_precision(
        reason="bf16 matmul pipeline; fp32 PSUM accumulate"
    ):
        _emit(nc, tc, with_bias)
    nc.compile()
    return nc


def make_in_maps(x, Wqkv, bqkv, Wproj, bproj):
    x = np.asarray(x, dtype=np.float32)
    Wqkv = np.asarray(Wqkv, dtype=np.float32)
    bqkv = np.asarray(bqkv, dtype=np.float32)
    Wproj = np.asarray(Wproj, dtype=np.float32)
    bproj = np.asarray(bproj, dtype=np.float32)
    keep = np.triu(np.ones((P, P), dtype=np.float32))  # [k, q]: k <= q
    cb = np.concatenate([
        keep,                                 # binary causal mask
        np.ones((P, HPG), dtype=np.float32),  # vones
    ], axis=1).astype(BF)
    in_maps = []
    for c in range(8):
        b, hg = c // 2, c % 2
        g = slice(hg * EG, (hg + 1) * EG)
        bqk = np.concatenate([
            (bqkv[0 * E:1 * E][g] * SCALE).reshape(CT, P).T,
            bqkv[1 * E:2 * E][g].reshape(CT, P).T], axis=1)
        crow = np.concatenate([
            np.ones(QCH, dtype=np.float32),
            bqkv[2 * E:3 * E][g],
            bproj if hg == 0 else np.zeros_like(bproj),
        ]).reshape(1, QCH + EG + E)
        def pack(a):
            # [n*128, m] -> [128, n*m] with row-major et-chunks
            n = a.shape[0] // P
            return np.ascontiguousarray(
                a.reshape(n, P, a.shape[1]).transpose(1, 0, 2)
                .reshape(P, n * a.shape[1])).astype(BF)
        in_maps.append({
            "xT": pack(x[b].T),
            "wq": pack(Wqkv[:, 0 * E:1 * E][:, g] * SCALE),
            "wk": pack(Wqkv[:, 1 * E:2 * E][:, g]),
            "wv": pack(Wqkv[:, 2 * E:3 * E][:, g]),
            "wp": pack(Wproj[g, :]),
            "cb": cb,
            "bqk": np.ascontiguousarray(bqk),
            "crow": np.ascontiguousarray(crow),
        })
    return in_maps


def kernel(x, Wqkv, bqkv, Wproj, bproj):
    with_bias = bool(
        np.any(np.asarray(bqkv)) or np.any(np.asarray(bproj)))
    nc = build_nc(with_bias)
    in_maps = make_in_maps(x, Wqkv, bqkv, Wproj, bproj)
    res = run_bass_kernel_spmd(nc, in_maps, list(range(8))).results
    out = np.zeros((B, S, E), dtype=np.float32)
    for c in range(8):
        out[c // 2] += res[c]["out"].astype(np.float32)
    return out
